# revision 1
# baseline (speedup 1.0000x reference)
"""Trainium2 Bass kernel for nn_ExpandEvecs.

Computes, for evecs [B=4, C=1, N=1024, K=16]:
    outers[b,k,i,j] = evecs[b,0,i,k] * evecs[b,0,j,k]
    cube = cumsum(outers, axis=k)  ->  [B, K, N, N]
i.e. cube[b,l] = V[:, :l+1] @ V[:, :l+1]^T  (Gram expansion per level).

Sharding: 8 cores = 4 batches x 2 row-halves. Core c (b=c//2, h=c%2)
computes all 16 levels for its 512-row half of batch b:
    out_c[l] = V[h*512:(h+1)*512, :l+1] @ V[:, :l+1]^T     [16, 512, 1024]
No inter-core communication. The 256 MiB f32 output (32 MiB/core) makes
this an HBM-write-bound problem (~94 us/core roofline at ~358 GB/s).

Precision/speed trick: split V = A + B with A = bf16(V), B = bf16(V-A).
Then V V^T ~= A A^T + A B^T + B A^T (the dropped B B^T term is ~2^-18
relative). All three terms are computed by ONE bf16 matmul per output
tile using partition-interleaved stacking with contraction K' = 3*(l+1):
    lhsT partitions (3k, 3k+1, 3k+2) = (A_k, A_k, B_k)   [row half]
    rhs  partitions (3k, 3k+1, 3k+2) = (A_k, B_k, A_k)   [all cols]
bf16 streams 1 col/cycle on the PE (vs ~2.5 for fp32r), and bf16
products are exact in the fp32 PSUM accumulator.

Per-core kernel: stacked operands live in SBUF ([48,1024]+[48,512] bf16,
loaded once); each (level, 128-row block) is 2 matmuls into a 2-bank
PSUM tile, a PSUM->SBUF copy split across the Vector and Scalar engines,
and one contiguous 512 KiB DMA store. Steady state is bound by the
16 SDMA engines' aggregate ~400 GB/s (4 KiB packets at ~164 ns).
"""

import numpy as np
import ml_dtypes

import concourse.mybir as mybir
from concourse import bacc, bass
from concourse.tile import TileContext
from concourse.bass_utils import run_bass_kernel_spmd

B, C, N, K = 4, 1, 1024, 16
NCORES = 8
HALF = N // 2          # rows per core
RB = HALF // 128       # 128-row blocks per core (4)
K3 = 3 * K             # stacked contraction partitions

F32 = mybir.dt.float32
BF16 = mybir.dt.bfloat16
BF16_NP = ml_dtypes.bfloat16

_nc_cache = None


def _build():
    nc = bacc.Bacc(None, target_bir_lowering=False)
    t_d = nc.declare_dram_parameter("t", [K3, N], BF16, isOutput=False)
    tl_d = nc.declare_dram_parameter("tl", [K3, HALF], BF16, isOutput=False)
    out_d = nc.declare_dram_parameter("out", [K, HALF, N], F32, isOutput=True)

    with TileContext(nc) as tc:
        with (
            tc.tile_pool(name="vpool", bufs=1) as vpool,
            tc.tile_pool(name="stage", bufs=8) as stage,
            tc.tile_pool(name="psum", bufs=4, space=bass.MemorySpace.PSUM) as psum,
        ):
            t = vpool.tile([K3, N], BF16)
            tl = vpool.tile([K3, HALF], BF16)
            t0 = vpool.tile([6, N], BF16)
            tl0 = vpool.tile([6, HALF], BF16)
            # tiny level-0/1 slices first (unblocks the first matmuls ~1us
            # earlier), then the full operands; two HWDGE rings in parallel
            nc.sync.dma_start(out=tl0[:], in_=tl_d[:6, :])
            nc.scalar.dma_start(out=t0[:], in_=t_d[:6, :])
            nc.sync.dma_start(out=tl[:], in_=tl_d[:])
            nc.scalar.dma_start(out=t[:], in_=t_d[:])

            # row-pair interleave: partition p of a 256-row group holds DRAM
            # rows 2p and 2p+1, so each partition's store run is 8 KiB
            # contiguous (halves DMA descriptor count). The lhsT for
            # sub-row r is a stride-2 slice of tl.
            tlv = tl.rearrange("k (m r) -> k m r", m=128, r=4)
            # levels 0-1: fine-grained 512 KiB stores to start the DMA
            # stream as early as possible during the ramp
            for lvl in range(2):
                kk = 3 * (lvl + 1)
                lhs_t, rhs_t = tl0, t0
                for i in range(RB):
                    ps = psum.tile([128, N], F32, tag="ps")
                    st = stage.tile([128, N], F32, tag="st")
                    for j in range(2):
                        nc.tensor.matmul(
                            ps[:, j * 512:(j + 1) * 512],
                            lhsT=lhs_t[:kk, i * 128:(i + 1) * 128],
                            rhs=rhs_t[:kk, j * 512:(j + 1) * 512],
                            start=True,
                            stop=True,
                        )
                    nc.vector.tensor_copy(st[:, :512], ps[:, :512])
                    nc.scalar.copy(st[:, 512:], ps[:, 512:])
                    nc.sync.dma_start(
                        out=out_d[lvl, i * 128:(i + 1) * 128, :], in_=st[:]
                    )

            # levels 2+: r=4 row interleave -> 16 KiB contiguous runs per
            # partition, one 2 MiB store per level
            for lvl in range(2, K):
                kk = 3 * (lvl + 1)  # stacked contraction size at this level
                st = stage.tile([128, 4, N], F32, tag="st")
                for r in range(4):
                    ps = psum.tile([128, N], F32, tag="ps")  # 2 banks
                    for j in range(2):
                        nc.tensor.matmul(
                            ps[:, j * 512:(j + 1) * 512],
                            lhsT=tlv[:kk, :, r],
                            rhs=t[:kk, j * 512:(j + 1) * 512],
                            start=True,
                            stop=True,
                        )
                    # copy each r-quarter as soon as its matmuls finish
                    if r % 2 == 0:
                        nc.vector.tensor_copy(st[:, r, :], ps[:])
                    else:
                        nc.scalar.copy(st[:, r, :], ps[:])
                nc.sync.dma_start(
                    out=out_d[lvl].rearrange("(p r) f -> p r f", p=128),
                    in_=st[:, :, :],
                )

    nc.compile()
    return nc


def _get_nc():
    global _nc_cache
    if _nc_cache is None:
        _nc_cache = _build()
    return _nc_cache


def _prepare_in_maps(evecs: np.ndarray) -> list[dict]:
    in_maps = []
    for c in range(NCORES):
        b, h = divmod(c, 2)
        vt = np.ascontiguousarray(evecs[b, 0].T, dtype=np.float32)  # [K, N]
        a32 = vt.astype(BF16_NP).astype(np.float32)
        a = a32.astype(BF16_NP)                       # hi part
        bb = (vt - a32).astype(BF16_NP)               # lo part
        t = np.empty((K3, N), dtype=BF16_NP)
        t[0::3] = a
        t[1::3] = bb
        t[2::3] = a
        sl = slice(h * HALF, (h + 1) * HALF)
        tl = np.empty((K3, HALF), dtype=BF16_NP)
        tl[0::3] = a[:, sl]
        tl[1::3] = a[:, sl]
        tl[2::3] = bb[:, sl]
        in_maps.append({"t": t, "tl": tl})
    return in_maps


def _assemble(results: list[dict]) -> np.ndarray:
    out = np.empty((B, K, N, N), dtype=np.float32)
    for c in range(NCORES):
        b, h = divmod(c, 2)
        out[b, :, h * HALF:(h + 1) * HALF, :] = results[c]["out"]
    return out.reshape(B, K * C, N, N)


def kernel(evecs) -> np.ndarray:
    evecs = np.asarray(evecs, dtype=np.float32)
    assert evecs.shape == (B, C, N, K), evecs.shape
    nc = _get_nc()
    in_maps = _prepare_in_maps(evecs)
    last_err = None
    for _attempt in range(3):
        try:
            r = run_bass_kernel_spmd(nc, in_maps, list(range(NCORES)))
            return _assemble(r.results)
        except Exception as e:  # transient NRT/device hiccups: retry
            last_err = e
    raise last_err



# revision 10
# speedup vs baseline: 2.0364x; 2.0364x over previous
"""Trainium2 Bass kernel for nn_ExpandEvecs.

Computes, for evecs [B=4, C=1, N=1024, K=16]:
    outers[b,k,i,j] = evecs[b,0,i,k] * evecs[b,0,j,k]
    cube = cumsum(outers, axis=k)  ->  [B, K, N, N]
i.e. cube[b,l] = V[:, :l+1] @ V[:, :l+1]^T  (Gram expansion per level).

This is an HBM-write-bound problem (the full f32 output is 256 MiB
against a 256 KiB input; HBM-per-NeuronCore is ~358 GB/s). The kernel
therefore minimizes device-side output bytes; the host only *moves*
data afterwards (dtype upconvert, strided scatter, symmetric mirror) —
all arithmetic happens on device.

 1. Every level matrix V V^T is symmetric: only the 36 upper-triangular
    128x128 blocks of each 8x8 block grid are computed and stored
    (56.25% of elements). The host mirrors the 28 strictly-upper blocks
    into the lower triangle.
 2. Outputs are stored as fp16 (2 bytes): quantization error ~2^-11
    relative, and the end-to-end rel error vs the f32 reference is
    ~3.5e-3 against the 2e-2 gate (inputs are bf16 on the PE).

Per-core bytes drop 32 MiB -> 9.4 MiB, i.e. a ~26 us DMA roofline.

Sharding (SPMD: one program, per-core differences in DATA only):
core c = 2b + g handles batch b and levels 8g..8g+7 (slot s = level
8g+s). All matmuls use contraction depth 16; the lhsT for slot s is a
host-prepared copy of bf16(V^T) with rows > level zero-masked, so the
accumulated Gram is truncated at the right rank while every core runs
identical shapes. Contraction depth is free on the PE (cost is
free-dim-bound), so the masking costs nothing.

Per slot: 8 row-strips (strip i = rows 128i..128(i+1), cols 128i..1024
of the triangle), each a run of <=512-col matmuls into 1-bank PSUM
tiles; PSUM is drained with f32->fp16 cast copies split across the
Vector and Scalar engines into a [128, 4608] staging tile (the strip
concat layout), then stored with two ~576 KiB contiguous DMAs.
"""

import numpy as np
import ml_dtypes

import concourse.mybir as mybir
from concourse import bacc, bass
from concourse.tile import TileContext
from concourse.bass_utils import run_bass_kernel_spmd

B, C, N, K = 4, 1, 1024, 16
NCORES = 8
NB = N // 128            # 8 block-rows
SLOTS = K // 2           # levels per core
STRIP_F = [(NB - i) * 128 for i in range(NB)]          # strip free sizes
OFF = [sum(STRIP_F[:i]) for i in range(NB)]            # concat offsets
TOT = sum(STRIP_F)                                     # 4608 cols per level

# Matmul chunks: cut the 4608-col concat stream at strip boundaries
# (lhsT changes), 1024-col PSUM-tile boundaries (drain granularity) and
# 512-col PSUM-bank boundaries within each tile (a matmul must not
# cross a bank). (strip, concat col, width) triples:
CHUNKS = []
for i in range(NB):
    o = OFF[i]
    end = OFF[i] + STRIP_F[i]
    while o < end:
        nxt = min(end, (o // 512 + 1) * 512)
        CHUNKS.append((i, o, nxt - o))
        o = nxt
# PSUM tiles: [1024k, 1024(k+1)) col ranges of the concat stream
PTILES = [(k * 1024, min(TOT, (k + 1) * 1024)) for k in range((TOT + 1023) // 1024)]

F32 = mybir.dt.float32
F16 = mybir.dt.float16
BF16 = mybir.dt.bfloat16
BF16_NP = ml_dtypes.bfloat16

_nc_cache = None


def _build():
    nc = bacc.Bacc(None, target_bir_lowering=False)
    t_d = nc.declare_dram_parameter("t", [K, N], BF16, isOutput=False)
    # slot-s zero-masked weights at cols [s*N, (s+1)*N) (free-dim packing:
    # matmul lhsT requires base partition 0)
    tl_d = nc.declare_dram_parameter("tl", [K, 8 * N], BF16, isOutput=False)
    out_d = nc.declare_dram_parameter("out", [SLOTS, 128, TOT], F16, isOutput=True)

    with TileContext(nc) as tc:
        with (
            tc.tile_pool(name="vpool", bufs=1) as vpool,
            tc.tile_pool(name="stage", bufs=3) as stage,
            tc.tile_pool(name="psum", bufs=4, space=bass.MemorySpace.PSUM) as psum,
        ):
            t = vpool.tile([K, N], BF16)
            tl0 = vpool.tile([K, N], BF16)      # slot-0 lhsT (early load)
            tl = vpool.tile([K, 8 * N], BF16)
            # rhs + slot-0 weights first so the first matmuls start ~1us
            # earlier; the full masked-weight stack follows.
            nc.scalar.dma_start(out=t[:], in_=t_d[:])
            nc.sync.dma_start(out=tl0[:], in_=tl_d[:, :N])
            nc.scalar.dma_start(out=tl[:], in_=tl_d[:])

            for s in range(SLOTS):
                lhs = tl0 if s == 0 else tl
                c0 = 0 if s == 0 else s * N
                st = stage.tile([128, TOT], F16, tag="st")
                for k, (lo, hi) in enumerate(PTILES):
                    ps = psum.tile([128, 1024], F32, tag="ps")
                    for (i, o, w) in CHUNKS:
                        if not (lo <= o < hi):
                            continue
                        ro = 128 * i + (o - OFF[i])  # rhs col
                        nc.tensor.matmul(
                            ps[:, o - lo:o - lo + w],
                            lhsT=lhs[:, c0 + 128 * i:c0 + 128 * (i + 1)],
                            rhs=t[:, ro:ro + w],
                            start=True,
                            stop=True,
                        )
                    # one f32->fp16 cast copy per PSUM tile (PSUM source
                    # caps DVE/ACT at 1 elem/cycle; amortize the per-op
                    # overhead). DVE: tiles 0+4, ACT: 1+3; tile 2
                    # alternates by slot to balance engine totals.
                    use_v = k in (0, 4) if k != 2 else (s % 2 == 0)
                    if use_v:
                        nc.vector.tensor_copy(st[:, lo:hi], ps[:, :hi - lo])
                    else:
                        nc.scalar.copy(st[:, lo:hi], ps[:, :hi - lo])
                    # stores after tiles 1, 3, 4: 512K/512K/128K chunks
                    if k in (1, 3, 4):
                        slo = {1: 0, 3: 2048, 4: 4096}[k]
                        nc.sync.dma_start(
                            out=out_d[s, :, slo:hi], in_=st[:, slo:hi]
                        )

    nc.compile()
    return nc


def _get_nc():
    global _nc_cache
    if _nc_cache is None:
        _nc_cache = _build()
    return _nc_cache


def _prepare_in_maps(evecs: np.ndarray) -> list[dict]:
    in_maps = []
    for c in range(NCORES):
        b, g = divmod(c, 2)
        vt = np.ascontiguousarray(evecs[b, 0].T, dtype=np.float32)  # [K, N]
        a = vt.astype(BF16_NP)
        tl = np.zeros((K, 8 * N), dtype=BF16_NP)
        for s in range(SLOTS):
            lvl = 8 * g + s
            tl[:lvl + 1, s * N:(s + 1) * N] = a[:lvl + 1]
        in_maps.append({"t": a, "tl": tl})
    return in_maps


def _assemble(results: list[dict]) -> np.ndarray:
    out = np.empty((B, K, N, N), dtype=np.float32)
    for c in range(NCORES):
        b, g = divmod(c, 2)
        buf = np.asarray(results[c]["out"]).astype(np.float32)  # [8,128,4608]
        lv = slice(8 * g, 8 * g + SLOTS)
        for i in range(NB):
            out[b, lv, 128 * i:128 * (i + 1), 128 * i:] = \
                buf[:, :, OFF[i]:OFF[i] + STRIP_F[i]]
    # mirror the strictly-upper 128x128 blocks into the lower triangle
    for i in range(NB):
        for j in range(i + 1, NB):
            out[:, :, 128 * j:128 * (j + 1), 128 * i:128 * (i + 1)] = \
                out[:, :, 128 * i:128 * (i + 1), 128 * j:128 * (j + 1)] \
                .transpose(0, 1, 3, 2)
    return out.reshape(B, K * C, N, N)


def kernel(evecs) -> np.ndarray:
    evecs = np.asarray(evecs, dtype=np.float32)
    assert evecs.shape == (B, C, N, K), evecs.shape
    nc = _get_nc()
    in_maps = _prepare_in_maps(evecs)
    last_err = None
    for _attempt in range(3):
        try:
            r = run_bass_kernel_spmd(nc, in_maps, list(range(NCORES)))
            return _assemble(r.results)
        except Exception as e:  # transient NRT/device hiccups: retry
            last_err = e
    raise last_err


# revision 13
# speedup vs baseline: 2.1516x; 1.0566x over previous
"""Trainium2 Bass kernel for nn_ExpandEvecs.

Computes, for evecs [B=4, C=1, N=1024, K=16]:
    outers[b,k,i,j] = evecs[b,0,i,k] * evecs[b,0,j,k]
    cube = cumsum(outers, axis=k)  ->  [B, K, N, N]
i.e. cube[b,l] = V[:, :l+1] @ V[:, :l+1]^T  (Gram expansion per level).

This is an HBM-write-bound problem (the full f32 output is 256 MiB
against a 256 KiB input; HBM-per-NeuronCore is ~358 GB/s). The kernel
therefore minimizes device-side output bytes; the host only *moves*
data afterwards (dtype upconvert, strided scatter, symmetric mirror) —
all arithmetic happens on device.

 1. Every level matrix V V^T is symmetric: only the 36 upper-triangular
    128x128 blocks of each 8x8 block grid are computed and stored
    (56.25% of elements). The host mirrors the 28 strictly-upper blocks
    into the lower triangle.
 2. Outputs are stored as fp16 (2 bytes): quantization error ~2^-11
    relative, and the end-to-end rel error vs the f32 reference is
    ~3.5e-3 against the 2e-2 gate (inputs are bf16 on the PE).

Per-core bytes drop 32 MiB -> 9.4 MiB, i.e. a ~26 us DMA roofline.

Sharding (SPMD: one program, per-core differences in DATA only):
core c = 2b + g handles batch b and levels 8g..8g+7 (slot s = level
8g+s). All matmuls use contraction depth 16; the lhsT for slot s is a
host-prepared copy of bf16(V^T) with rows > level zero-masked, so the
accumulated Gram is truncated at the right rank while every core runs
identical shapes. Contraction depth is free on the PE (cost is
free-dim-bound), so the masking costs nothing.

Per slot: 8 row-strips (strip i = rows 128i..128(i+1), cols 128i..1024
of the triangle), each a run of <=512-col matmuls into 1-bank PSUM
tiles; PSUM is drained with f32->fp16 cast copies split across the
Vector and Scalar engines into a [128, 4608] staging tile (the strip
concat layout), then stored with two ~576 KiB contiguous DMAs.
"""

import numpy as np
import ml_dtypes

import concourse.mybir as mybir
from concourse import bacc, bass
from concourse.tile import TileContext
from concourse.bass_utils import run_bass_kernel_spmd

B, C, N, K = 4, 1, 1024, 16
NCORES = 8
NB = N // 128            # 8 block-rows
SLOTS = K // 2           # levels per core
STRIP_F = [(NB - i) * 128 for i in range(NB)]          # strip free sizes
OFF = [sum(STRIP_F[:i]) for i in range(NB)]            # concat offsets
TOT = sum(STRIP_F)                                     # 4608 cols per level

# Matmul chunks: cut the 4608-col concat stream at strip boundaries
# (lhsT changes), 1024-col PSUM-tile boundaries (drain granularity) and
# 512-col PSUM-bank boundaries within each tile (a matmul must not
# cross a bank). (strip, concat col, width) triples:
CHUNKS = []
for i in range(NB):
    o = OFF[i]
    end = OFF[i] + STRIP_F[i]
    while o < end:
        nxt = min(end, (o // 512 + 1) * 512)
        CHUNKS.append((i, o, nxt - o))
        o = nxt
# PSUM tiles: [1024k, 1024(k+1)) col ranges of the concat stream
PTILES = [(k * 1024, min(TOT, (k + 1) * 1024)) for k in range((TOT + 1023) // 1024)]

F32 = mybir.dt.float32
F16 = mybir.dt.float16
BF16 = mybir.dt.bfloat16
BF16_NP = ml_dtypes.bfloat16

_nc_cache = None


def _build():
    nc = bacc.Bacc(None, target_bir_lowering=False)
    t_d = nc.declare_dram_parameter("t", [K, N], BF16, isOutput=False)
    # slot-s zero-masked weights at cols [s*N, (s+1)*N) (free-dim packing:
    # matmul lhsT requires base partition 0)
    tl_d = nc.declare_dram_parameter("tl", [K, 8 * N], BF16, isOutput=False)
    out_d = nc.declare_dram_parameter("out", [SLOTS, 128, TOT], F16, isOutput=True)

    with TileContext(nc) as tc:
        with (
            tc.tile_pool(name="vpool", bufs=1) as vpool,
            tc.tile_pool(name="stage", bufs=3) as stage,
            tc.tile_pool(name="psum", bufs=4, space=bass.MemorySpace.PSUM) as psum,
        ):
            # weights/rhs duplicated at partitions 0:16 and 64:80 -> two
            # PE row groups (tile_position (0,0) / (64,0)) stream matmuls
            # concurrently (contraction 16 <= 32), ~2x PE throughput.
            t = vpool.tile([128, N], BF16)
            tl0 = vpool.tile([128, N], BF16)    # slot-0 lhsT (early load)
            tl = vpool.tile([128, 8 * N], BF16)
            # rhs + slot-0 weights first so the first matmuls start ~1us
            # earlier; the full masked-weight stack follows.
            for g, eng in ((0, nc.scalar), (64, nc.sync)):
                eng.dma_start(out=t[g:g + K, :], in_=t_d[:])
                eng.dma_start(out=tl0[g:g + K, :], in_=tl_d[:, :N])
                eng.dma_start(out=tl[g:g + K, :], in_=tl_d[:])

            for s in range(SLOTS):
                lhs = tl0 if s == 0 else tl
                c0 = 0 if s == 0 else s * N
                st = stage.tile([128, TOT], F16, tag="st")
                for k, (lo, hi) in enumerate(PTILES):
                    ps = psum.tile([128, 1024], F32, tag="ps")
                    for (i, o, w) in CHUNKS:
                        if not (lo <= o < hi):
                            continue
                        ro = 128 * i + (o - OFF[i])  # rhs col
                        # PE row group = PSUM bank parity: concurrent
                        # cross-group matmuls never share a PSUM bank
                        g = 64 * ((o // 512) % 2)
                        nc.tensor.matmul(
                            ps[:, o - lo:o - lo + w],
                            lhsT=lhs[g:g + K, c0 + 128 * i:c0 + 128 * (i + 1)],
                            rhs=t[g:g + K, ro:ro + w],
                            start=True,
                            stop=True,
                        )
                    # one f32->fp16 cast copy per PSUM tile (PSUM source
                    # caps DVE/ACT at 1 elem/cycle; amortize the per-op
                    # overhead). DVE: tiles 0+4, ACT: 1+3; tile 2
                    # alternates by slot to balance engine totals.
                    use_v = k in (0, 4) if k != 2 else (s % 2 == 0)
                    if use_v:
                        nc.vector.tensor_copy(st[:, lo:hi], ps[:, :hi - lo])
                    else:
                        nc.scalar.copy(st[:, lo:hi], ps[:, :hi - lo])
                    # stores after tiles 1, 3, 4: 512K/512K/128K chunks
                    if k in (1, 3, 4):
                        slo = {1: 0, 3: 2048, 4: 4096}[k]
                        nc.sync.dma_start(
                            out=out_d[s, :, slo:hi], in_=st[:, slo:hi]
                        )

    nc.compile()
    return nc


def _get_nc():
    global _nc_cache
    if _nc_cache is None:
        _nc_cache = _build()
    return _nc_cache


def _prepare_in_maps(evecs: np.ndarray) -> list[dict]:
    in_maps = []
    for c in range(NCORES):
        b, g = divmod(c, 2)
        vt = np.ascontiguousarray(evecs[b, 0].T, dtype=np.float32)  # [K, N]
        a = vt.astype(BF16_NP)
        tl = np.zeros((K, 8 * N), dtype=BF16_NP)
        for s in range(SLOTS):
            lvl = 8 * g + s
            tl[:lvl + 1, s * N:(s + 1) * N] = a[:lvl + 1]
        in_maps.append({"t": a, "tl": tl})
    return in_maps


def _assemble(results: list[dict]) -> np.ndarray:
    out = np.empty((B, K, N, N), dtype=np.float32)
    for c in range(NCORES):
        b, g = divmod(c, 2)
        buf = np.asarray(results[c]["out"]).astype(np.float32)  # [8,128,4608]
        lv = slice(8 * g, 8 * g + SLOTS)
        for i in range(NB):
            out[b, lv, 128 * i:128 * (i + 1), 128 * i:] = \
                buf[:, :, OFF[i]:OFF[i] + STRIP_F[i]]
    # mirror the strictly-upper 128x128 blocks into the lower triangle
    for i in range(NB):
        for j in range(i + 1, NB):
            out[:, :, 128 * j:128 * (j + 1), 128 * i:128 * (i + 1)] = \
                out[:, :, 128 * i:128 * (i + 1), 128 * j:128 * (j + 1)] \
                .transpose(0, 1, 3, 2)
    return out.reshape(B, K * C, N, N)


def kernel(evecs) -> np.ndarray:
    evecs = np.asarray(evecs, dtype=np.float32)
    assert evecs.shape == (B, C, N, K), evecs.shape
    nc = _get_nc()
    in_maps = _prepare_in_maps(evecs)
    last_err = None
    for _attempt in range(3):
        try:
            r = run_bass_kernel_spmd(nc, in_maps, list(range(NCORES)))
            return _assemble(r.results)
        except Exception as e:  # transient NRT/device hiccups: retry
            last_err = e
    raise last_err
